# revision 27
# baseline (speedup 1.0000x reference)
"""Trainium2 Bass kernel for the span-search problem (nn_DCR_21285857919673).

Data-parallel over batch: 32 batches / 8 cores = 4 per core. The host ships
seq pre-transposed ([h, token]), compacted to the valid token span
(sep0+1 .. sep1), and split losslessly into an fp16 hi/lo pair (same 4B/elem
of DMA as fp32, but PE fp16 matmuls run 4x faster than fp32). Batches are
globally sorted by span into 4 width slots (one batch per slot per core) so
the compiled widths are minimal; each core processes its slots
largest-first.

Per slot, per h-chunk (8 chunks of 128):
  PE:  d1,d2 via 3 fp16 matmul passes (hi*qhi + hi*qlo + lo*qhi) -> fp32 PSUM
       (error ~2^-22: full-fp32 quality for the argmax ties), in 512-col
       bank-aligned pieces; n2 via fp32r ones-matmuls over Act-squared hi
       (n2 only needs ~5e-5 relative accuracy -- it enters through
       sqrt + ratio -- so the tf32-grade fp32r path is safe there).
Then d/n2 rows go to DRAM scratch and the banded window stage
(overlapping-AP gathers, masked max / first-argmax) runs, split across
DVE/Pool/Act. Slot k's banded stage is emitted inside slot k+1's chunk loop
so only the last (smallest) slot's banded work trails the matmul phase.
"""
import sys

sys.path.insert(0, "/opt/trn_rl_repo")

import numpy as np

import concourse.bass as bass
import concourse.bacc as bacc
import concourse.bass_isa as bass_isa
import concourse.mybir as mybir
import concourse.tile as tile
from concourse.alu_op_type import AluOpType
from concourse.bass_utils import run_bass_kernel_spmd

F32 = mybir.dt.float32
F32R = mybir.dt.float32r
F16 = mybir.dt.float16
I32 = mybir.dt.int32
U8 = mybir.dt.uint8

B = 32
S = 1024
H = 1024
L = 32
NC = H // 128
NCORES = 8
NEG = -10000.0
PAD_VAL = 0.25

CHUNK_GROUPS = [[0], [1, 2, 3], [4, 5], [6, 7]]

_cache = {}


def _pieces_bank(w):
    return [(off, min(512, w - off)) for off in range(0, w, 512)]


def _pieces_balanced(w):
    """Even-length pieces <=512 (fp32r needs even cols, >=256 for full rate)."""
    assert w % 2 == 0
    n = max(1, (w + 511) // 512)
    base = (w // n) & ~1
    out = []
    off = 0
    for i in range(n):
        ln = base if i < n - 1 else w - off
        out.append((off, ln))
        off += ln
    assert all(ln % 2 == 0 and ln <= 512 for _, ln in out)
    return out


def _build(W, NT):
    """W: slot widths [4] (even, desc), NT: ceil(W/128)."""
    NT_MAX = max(NT)
    SP = NT_MAX * 128 + 64
    LU = sum(NT[k] * L for k in range(4))
    LI = sum(NT[k] for k in range(4))
    nc = bacc.Bacc("TRN2", target_bir_lowering=False, debug=False)

    # partition-major; per chunk: [hi (W) | lo (W)]
    hilos = [nc.dram_tensor(f"hilo{k}", [128, NC * 2 * W[k]], F16,
                            kind="ExternalInput").ap() for k in range(4)]
    # qh block then ql block, each [128, NC*4*2]: (c, slot) -> 2 cols
    qhl_in = nc.dram_tensor("qhl", [128, 2 * NC * 4 * 2], F16, kind="ExternalInput").ap()
    vpack_in = nc.dram_tensor("vpack", [128, LU + LI], U8, kind="ExternalInput").ap()
    fpack_in = nc.dram_tensor("fpack", [128, NT_MAX * L + LI + 4 * 16], F32,
                              kind="ExternalInput").ap()
    ones_in = nc.dram_tensor("ones", [128, 2], F32R, kind="ExternalInput").ap()

    mvei_out = nc.dram_tensor("mvei", [4, 2 * NT_MAX * 128], F32, kind="ExternalOutput").ap()
    scratch = nc.dram_tensor("scratch", [4, 3, SP], F32).ap()

    with tile.TileContext(nc) as tc:
        with (
            tc.tile_pool(name="consts", bufs=1) as consts,
            tc.tile_pool(name="seqp", bufs=4) as seqp,
            tc.tile_pool(name="sqp", bufs=3) as sqp,
            tc.tile_pool(name="rows", bufs=2) as rows_p,
            tc.tile_pool(name="band", bufs=2) as band_p,
            tc.tile_pool(name="pd", bufs=2, space="PSUM") as pd,
            tc.tile_pool(name="pn", bufs=2, space="PSUM") as pn,
        ):
            # qhl gates the first matmul: ship it first (tiny)
            c_qhl = consts.tile([128, 2 * NC * 4 * 2], F16, tag="qhl")
            nc.sync.dma_start(c_qhl[:], qhl_in)
            c_ones = consts.tile([128, 2], F32R, tag="ones")
            nc.sync.dma_start(c_ones[:], ones_in)

            slot_tiles = {}

            def emit_chunk_dmas(k):
                tiles = []
                w2 = 2 * W[k]
                for g in CHUNK_GROUPS:
                    t = seqp.tile([128, len(g) * w2], F16, tag=f"hilo{len(g)}",
                                  name=f"hilo_s{k}_g{g[0]}")
                    nc.sync.dma_start(t[:], hilos[k][:, g[0] * w2:(g[-1] + 1) * w2])
                    for gi, c in enumerate(g):
                        tiles.append((t, gi))
                slot_tiles[k] = tiles

            emit_chunk_dmas(0)

            c_vpack = consts.tile([128, LU + LI], U8, tag="vpack")
            nc.sync.dma_start(c_vpack[:], vpack_in)
            c_fpack = consts.tile([128, NT_MAX * L + LI + 4 * 16], F32, tag="fpack")
            nc.sync.dma_start(c_fpack[:], fpack_in)

            emit_chunk_dmas(1)
            emit_chunk_dmas(2)

            # const views
            off_u = [0]
            for k in range(4):
                off_u.append(off_u[-1] + NT[k] * L)
            vmj = [c_vpack[:, off_u[k]:off_u[k + 1]] for k in range(4)]
            off_i = LU
            vmi = []
            for k in range(4):
                vmi.append(c_vpack[:, off_i:off_i + NT[k]])
                off_i += NT[k]
            riota = c_fpack[:, 0:NT_MAX * L]
            off_f = NT_MAX * L
            cconst = []
            for k in range(4):
                cconst.append(c_fpack[:, off_f:off_f + NT[k]])
                off_f += NT[k]
            qcat = c_fpack[:, off_f:off_f + 4 * 16]

            c_negL = consts.tile([128, NT_MAX * L], F32, tag="negL")
            nc.vector.memset(c_negL[:], NEG)
            c_neg1 = consts.tile([128, NT_MAX], F32, tag="neg1")
            nc.vector.memset(c_neg1[:], -1.0)

            qpart = consts.tile([128, 4], F32, tag="qpart")
            qtrash = consts.tile([128, 16], F32, tag="qtrash")
            qn2all = consts.tile([128, 4], F32, tag="qn2all")
            qsq = consts.tile([128, 4], F32, tag="qsq")
            rsqall = consts.tile([128, 4], F32, tag="rsqall")

            def emit_qn2():
                for k in range(4):
                    nc.scalar.activation(qtrash[:], qcat[:, k * 16:(k + 1) * 16],
                                         mybir.ActivationFunctionType.Square,
                                         accum_out=qpart[:, k:k + 1])
                nc.gpsimd.partition_all_reduce(qn2all[:], qpart[:], 128,
                                               bass_isa.ReduceOp.add)
                nc.scalar.sqrt(qsq[:], qn2all[:])
                nc.vector.reciprocal(rsqall[:], qsq[:])

            def bcast_l(ap_col, nt):
                return bass.AP(ap_col.tensor, ap_col.offset,
                               [[ap_col.ap[0][0], 128], [ap_col.ap[-1][0], nt], [0, L]])

            def win_col(tile_, nt):
                a = tile_[:]
                return bass.AP(tile_.tensor, a.offset,
                               [[a.ap[0][0], 128], [a.ap[1][0], nt], [0, L]])

            def rearr(ap_, nt):
                return bass.AP(ap_.tensor, ap_.offset,
                               [[ap_.ap[0][0], 128], [L, nt], [1, L]])

            def emit_phase_a(k, mid=None):
                w = W[k]
                dpieces = _pieces_bank(w)
                npieces = _pieces_balanced(w)
                dps = pd.tile([2, w], F32, tag="dps", name=f"dps{k}")
                ntile = [pn.tile([1, 512], F32, tag=f"n{j}", name=f"n{j}_{k}")
                         for j in range(len(npieces))]
                sq_tiles = []
                for c in range(NC):
                    t, gi = slot_tiles[k][c]
                    w2 = 2 * w
                    hi_c = t[:, gi * w2: gi * w2 + w]
                    lo_c = t[:, gi * w2 + w: (gi + 1) * w2]
                    qh_sl = c_qhl[:, (c * 4 + k) * 2:(c * 4 + k) * 2 + 2]
                    ql_sl = c_qhl[:, NC * 8 + (c * 4 + k) * 2:NC * 8 + (c * 4 + k) * 2 + 2]
                    for off, ln in dpieces:
                        nc.tensor.matmul(dps[:, off:off + ln], lhsT=qh_sl,
                                         rhs=hi_c[:, off:off + ln],
                                         start=(c == 0), stop=False)
                    for off, ln in dpieces:
                        nc.tensor.matmul(dps[:, off:off + ln], lhsT=qh_sl,
                                         rhs=lo_c[:, off:off + ln],
                                         start=False, stop=False)
                    for off, ln in dpieces:
                        nc.tensor.matmul(dps[:, off:off + ln], lhsT=ql_sl,
                                         rhs=hi_c[:, off:off + ln],
                                         start=False, stop=(c == NC - 1))
                    sq_c = sqp.tile([128, w], F32R, tag="sq", name=f"sq{k}_{c}")
                    nc.scalar.activation(sq_c[:], hi_c,
                                         mybir.ActivationFunctionType.Square)
                    sq_tiles.append(sq_c)
                    for cc2 in ([c - 1] if c > 0 else []) + ([c] if c == NC - 1 else []):
                        for j, (off, ln) in enumerate(npieces):
                            nc.tensor.matmul(ntile[j][0:1, 0:ln], lhsT=c_ones[:, 0:1],
                                             rhs=sq_tiles[cc2][:, off:off + ln],
                                             start=(cc2 == 0), stop=(cc2 == NC - 1))
                    if k == 0 and c == NC - 1:
                        emit_qn2()
                    if mid is not None and c == 2:
                        mid()
                return dps, ntile, npieces

            def emit_drain(k, dps, ntile, npieces):
                w = W[k]
                wk = NT[k] * 128 + 32
                dsb = rows_p.tile([2, wk + 8], F32, tag="dsb", name=f"dsb{k}")
                nc.vector.tensor_copy(dsb[:, 0:w], dps[:])
                n2sb = rows_p.tile([1, wk + 8], F32, tag="n2sb", name=f"n2sb{k}")
                for j, (off, ln) in enumerate(npieces):
                    nc.vector.tensor_copy(n2sb[0:1, off:off + ln], ntile[j][0:1, 0:ln])
                # width covers the window pad region [W, NT*128+32) with
                # whatever finite/NaN garbage the tiles hold -- it is masked.
                nc.sync.dma_start(
                    bass.AP(scratch.tensor, k * 3 * SP, [[SP, 2], [1, wk]]),
                    dsb[:, 0:wk])
                nc.sync.dma_start(
                    bass.AP(scratch.tensor, (k * 3 + 2) * SP, [[1, 1], [1, wk]]),
                    n2sb[0:1, 0:wk])

            def emit_banded(k):
                nt = NT[k]
                soff = k * 3 * SP
                d1col = band_p.tile([128, nt], F32, tag="d1col", name=f"d1c{k}")
                nc.sync.dma_start(d1col[:], bass.AP(scratch.tensor, soff,
                                                    [[1, 128], [128, nt]]))
                d2w = band_p.tile([128, nt, L], F32, tag="d2w", name=f"d2w{k}")
                nc.sync.dma_start(d2w[:], bass.AP(scratch.tensor, soff + SP,
                                                  [[1, 128], [128, nt], [1, L]]))
                n2w = band_p.tile([128, nt, L], F32, tag="n2w", name=f"n2w{k}")
                nc.sync.dma_start(n2w[:], bass.AP(scratch.tensor, soff + 2 * SP,
                                                  [[1, 128], [128, nt], [1, L]]))

                numer = band_p.tile([128, nt, L], F32, tag="numer", name=f"nu{k}")
                nc.gpsimd.tensor_tensor(out=numer[:], in0=d2w[:],
                                        in1=bcast_l(d1col[:], nt), op=AluOpType.add)
                nsum = band_p.tile([128, nt, L], F32, tag="nsum", name=f"ns{k}")
                nc.vector.tensor_tensor(out=nsum[:], in0=n2w[:], in1=win_col(n2w, nt),
                                        op=AluOpType.add)
                den = band_p.tile([128, nt, L], F32, tag="den", name=f"de{k}")
                nc.scalar.sqrt(den[:], nsum[:])
                rec = band_p.tile([128, nt, L], F32, tag="rec", name=f"re{k}")
                nc.vector.reciprocal(rec[:], den[:])
                sim = band_p.tile([128, nt, L], F32, tag="sim", name=f"si{k}")
                nc.vector.tensor_tensor(out=sim[:], in0=numer[:], in1=rec[:],
                                        op=AluOpType.mult)
                simm = band_p.tile([128, nt, L], F32, tag="simm", name=f"sm{k}")
                nc.vector.select(simm[:], rearr(vmj[k], nt), sim[:],
                                 rearr(c_negL[:, 0:nt * L], nt))
                maxv = band_p.tile([128, nt], F32, tag="maxv", name=f"mx{k}")
                nc.vector.tensor_reduce(out=maxv[:], in_=simm[:],
                                        axis=mybir.AxisListType.X, op=AluOpType.max)
                eq = band_p.tile([128, nt, L], F32, tag="eq", name=f"eq{k}")
                nc.vector.tensor_tensor(out=eq[:], in0=simm[:],
                                        in1=bcast_l(maxv[:], nt), op=AluOpType.is_equal)
                wt = band_p.tile([128, nt, L], F32, tag="wt", name=f"wq{k}")
                nc.gpsimd.tensor_tensor(out=wt[:], in0=eq[:],
                                        in1=rearr(riota[:, 0:nt * L], nt),
                                        op=AluOpType.mult)
                mval = band_p.tile([128, nt], F32, tag="mval", name=f"mv{k}")
                nc.vector.tensor_reduce(out=mval[:], in_=wt[:],
                                        axis=mybir.AxisListType.X, op=AluOpType.max)
                mvei = band_p.tile([128, 2 * nt], F32, tag="mvei", name=f"me{k}")
                nc.gpsimd.tensor_tensor(out=mvei[:, nt:2 * nt], in0=cconst[k],
                                        in1=mval[:], op=AluOpType.subtract)
                nc.vector.tensor_scalar(out=mvei[:, 0:nt], in0=maxv[:],
                                        scalar1=rsqall[:, k:k + 1], scalar2=None,
                                        op0=AluOpType.mult)
                # vmi ships inverted (1 = invalid start)
                nc.vector.copy_predicated(mvei[:, 0:nt], vmi[k], c_negL[:, 0:nt])
                nc.vector.copy_predicated(mvei[:, nt:2 * nt], vmi[k], c_neg1[:, 0:nt])
                nc.sync.dma_start(
                    bass.AP(mvei_out.tensor, k * 2 * NT_MAX * 128,
                            [[2 * nt, 128], [1, 2 * nt]]), mvei[:])

            state = {}

            def mk_mid(k):
                def mid():
                    dps, ntile, npieces = state[k]
                    emit_drain(k, dps, ntile, npieces)
                    emit_banded(k)
                return mid

            for k in range(4):
                if k == 2:
                    emit_chunk_dmas(3)
                state[k] = emit_phase_a(k, mid=mk_mid(k - 1) if k > 0 else None)
            emit_drain(3, *state[3])
            emit_banded(3)

    nc.compile()
    return nc


def _prep_core(seq, idx, order_c, W, NT):
    NT_MAX = max(NT)
    LU = sum(NT[k] * L for k in range(4))
    LI = sum(NT[k] for k in range(4))
    p128 = np.arange(128)
    qh = np.zeros((128, NC * 4 * 2), np.float16)
    ql = np.zeros((128, NC * 4 * 2), np.float16)
    qcat = np.zeros((128, 4 * 16), np.float32)
    im = {}
    spans = {}
    for k in range(4):
        w = W[k]
        b = order_c[k]
        sep0, sep1 = int(idx[b, 0]), int(idx[b, 1])
        span = max(0, sep1 - sep0 - 1)
        spans[k] = span
        hilo = np.full((NC, 128, 2 * w), PAD_VAL, np.float16)
        hilo[:, :, w:] = 0.0
        x = np.ascontiguousarray(seq[b, sep0 + 1:sep0 + 1 + span, :].T)
        xh = x.astype(np.float16)
        xl = (x - xh.astype(np.float32)).astype(np.float16)
        hilo[:, :, 0:span] = xh.reshape(NC, 128, span)
        hilo[:, :, w:w + span] = xl.reshape(NC, 128, span)
        im[f"hilo{k}"] = np.ascontiguousarray(
            hilo.transpose(1, 0, 2).reshape(128, NC * 2 * w))
        q1 = seq[b, 1, :]
        q2 = seq[b, max(sep0 - 1, 0), :]
        q1h, q2h = q1.astype(np.float16), q2.astype(np.float16)
        q1l = (q1 - q1h.astype(np.float32)).astype(np.float16)
        q2l = (q2 - q2h.astype(np.float32)).astype(np.float16)
        for c in range(NC):
            base = (c * 4 + k) * 2
            sl = slice(c * 128, (c + 1) * 128)
            qh[:, base] = q1h[sl]
            qh[:, base + 1] = q2h[sl]
            ql[:, base] = q1l[sl]
            ql[:, base + 1] = q2l[sl]
        qcat[:, k * 16:k * 16 + 8] = q1.reshape(128, 8, order="F")
        qcat[:, k * 16 + 8:k * 16 + 16] = q2.reshape(128, 8, order="F")
    vpack = np.zeros((128, LU + LI), np.uint8)
    fpack = np.zeros((128, NT_MAX * L + LI + 4 * 16), np.float32)
    off_u, off_i, off_f = 0, LU, NT_MAX * L
    fpack[:, 0:NT_MAX * L] = np.broadcast_to(
        (L - np.arange(L))[None, None, :], (128, NT_MAX, L)).reshape(128, NT_MAX * L)
    for k in range(4):
        b = order_c[k]
        sep0 = int(idx[b, 0])
        span = spans[k]
        nt = NT[k]
        i_comp = p128[:, None] + 128 * np.arange(nt)[None, :]
        jv = i_comp[:, :, None] + np.arange(L)[None, None, :]
        vpack[:, off_u:off_u + nt * L] = (jv < span).astype(np.uint8).reshape(128, nt * L)
        off_u += nt * L
        vpack[:, off_i:off_i + nt] = (i_comp >= span).astype(np.uint8)
        off_i += nt
        fpack[:, off_f:off_f + nt] = (sep0 + 1 + i_comp + L).astype(np.float32)
        off_f += nt
    fpack[:, off_f:off_f + 4 * 16] = qcat
    im["vpack"] = vpack
    im["fpack"] = fpack
    im["qhl"] = np.concatenate([qh, ql], axis=1)
    im["ones"] = np.ones((128, 2), np.float32)
    return im, spans


def kernel(sequence_outputs, idxs, max_ans_len):
    seq = np.asarray(sequence_outputs, dtype=np.float32)
    idx = np.asarray(idxs).astype(np.int64)
    assert int(max_ans_len) == L and seq.shape == (B, S, H)

    spans_all = np.maximum(idx[:, 1] - idx[:, 0] - 1, 0)
    order = np.argsort(-spans_all, kind="stable")
    W = [max(2, (int(spans_all[order[k * NCORES]]) + 1) & ~1) for k in range(4)]
    NT = [(w + 127) // 128 for w in W]

    key = (tuple(W),)
    if key not in _cache:
        _cache[key] = _build(W, NT)
    nc = _cache[key]

    NT_MAX = max(NT)
    in_maps, span_list = [], []
    for c in range(NCORES):
        order_c = [int(order[k * NCORES + c]) for k in range(4)]
        im, spans = _prep_core(seq, idx, order_c, W, NT)
        in_maps.append(im)
        span_list.append((order_c, spans))

    res = run_bass_kernel_spmd(nc, in_maps, core_ids=list(range(NCORES))).results

    mv = np.full((B, S), NEG, np.float32)
    ei = np.full((B, S), -1, np.int32)
    for c in range(NCORES):
        order_c, spans = span_list[c]
        for k in range(4):
            b = order_c[k]
            sep0 = int(idx[b, 0])
            span = spans[k]
            if span <= 0:
                continue
            nt = NT[k]
            flat = res[c]["mvei"][k, 0:128 * 2 * nt].reshape(128, 2 * nt)
            mvd = flat[:, 0:nt].T.ravel()
            eid = flat[:, nt:2 * nt].T.ravel()
            mv[b, sep0 + 1:sep0 + 1 + span] = mvd[0:span]
            ei[b, sep0 + 1:sep0 + 1 + span] = np.rint(eid[0:span]).astype(np.int32)
    return mv, ei
